# revision 13
# baseline (speedup 1.0000x reference)
"""Trainium2 Bass kernel for the MERU-style hyperbolic contrastive loss.

Problem (hardcoded shapes):
  text_embeddings (8192, 768) f32, label_embeddings (4096, 768) f32,
  target_labels (8192,) int32, three scalar log-params.
  Output: loss (8192,) f32 per-sample.

Sharding: data-parallel over text rows across 8 NeuronCores (1024 rows each);
label_embeddings and scalars replicated.

Per-core algorithm (factored so the PE does nearly all the math):
  The Lorentz inner product between hyperboloid points factors as
      inner[m,c] = hx_m * g_c * ( S_raw[m,c] - (xt_m/hx_m)*(yt_c/g_c) )
  where S_raw = raw_text @ raw_labels^T, hx_m = alpha_t*fac_m and
  g_c = alpha_l*fac_c are the exp-map scale factors, and xt/yt the time
  components.  So:
   1. Cast RAW text/labels to bf16, round-trip through DRAM with an XBAR
      DMA-transpose for the K-major PE layout (independent of all stats).
   2. Batched stats: per-row norms -> one wide ACT chain (Sqrt/Exp/Ln) for
      fac, time components, and the ratios xt/hx, yt/g (1/x via Exp(-Ln x));
      ratios split into exact bf16 hi+lo pairs and folded into the matmul
      as a K=3 rank term, so PSUM accumulates S_raw - (xt/hx)(yt/g).
   3. Per 128-row m-tile: 8 x (6 bf16 K=128 matmuls + 1 K=3 matmul) per
      PSUM bank; one scalar_tensor_tensor per bank rescales by hx_m (per-
      partition) * g_c (broadcast row) while copying PSUM -> SBUF.
   4. VectorE max (top-8 per row); the positive logit is extracted with an
      iota==target scalar_tensor_tensor accumulation (bitwise-identical to
      the matrix value) and removed from the candidates via match_replace;
      the re-sorted top-2 are the hardest negatives.
   5. Batched f32 tail over all m-tiles: dist = arccosh(max(-curv*v, 1+eps))
      / sqrt(curv) via Sqrt/Ln, loss = log(sum exp(-dist)) + pos_dist.
  Only ucode-safe op variants are used (no reciprocal/divide/TTR, no
  AP-scale activations).  End-to-end error vs the f32 reference ~1.5e-4.
"""

from contextlib import ExitStack

import numpy as np

import concourse.bass as bass
import concourse.tile as tile
from concourse import bacc, mybir
from concourse import bass_utils

F32 = mybir.dt.float32
BF16 = mybir.dt.bfloat16
I32 = mybir.dt.int32
AF = mybir.ActivationFunctionType
ALU = mybir.AluOpType
AX = mybir.AxisListType

N_CORES = 8
M_FULL = 8192
C = 4096
D = 768
M_LOC = M_FULL // N_CORES   # 1024 rows per core
P = 128
NT_M = M_LOC // P           # 8 m-tiles
NT_C = C // P               # 32 label tiles
KCH = D // P                # 6 contraction chunks
NB = C // 512               # 8 PSUM-bank n-chunks
NF = 512
EPS = 1e-8
BIG = 1e30


def _stats_chain(nc, pool, nsq_raw, w, alpha_b, curv_b, icurv_b, tg):
    """From ||raw_row||^2 (128,w) compute gfac = alpha*sinh(rc)/rc and
    tq = time/gfac, using only wide ACT ops and tensor_scalar/tensor ops.
    rc = sqrt(curv)*alpha*||raw_row||."""
    a2 = pool.tile([P, 1], F32, tag=f"a2{tg}")
    nc.vector.tensor_mul(a2[:], alpha_b[:], alpha_b[:])
    nsq = pool.tile([P, w], F32, tag=f"nsq{tg}")
    nc.vector.tensor_scalar(nsq[:], nsq_raw[:], a2[:], None, op0=ALU.mult)
    rc2 = pool.tile([P, w], F32, tag=f"rc2{tg}")
    nc.vector.tensor_scalar(rc2[:], nsq[:], curv_b[:], None, op0=ALU.mult)
    rc = pool.tile([P, w], F32, tag=f"rc{tg}")
    nc.scalar.activation(rc[:], rc2[:], AF.Sqrt)
    ep = pool.tile([P, w], F32, tag=f"ep{tg}")
    nc.scalar.activation(ep[:], rc[:], AF.Exp)
    en = pool.tile([P, w], F32, tag=f"en{tg}")
    nc.scalar.activation(en[:], rc[:], AF.Exp, scale=-1.0)
    sh = pool.tile([P, w], F32, tag=f"sh{tg}")
    nc.vector.tensor_sub(sh[:], ep[:], en[:])
    nc.vector.tensor_scalar_mul(sh[:], sh[:], 0.5)
    rcc = pool.tile([P, w], F32, tag=f"rcc{tg}")
    nc.vector.tensor_scalar_max(rcc[:], rc[:], EPS)
    lnr = pool.tile([P, w], F32, tag=f"lnr{tg}")
    nc.scalar.activation(lnr[:], rcc[:], AF.Ln)
    rinv = pool.tile([P, w], F32, tag=f"rinv{tg}")
    nc.scalar.activation(rinv[:], lnr[:], AF.Exp, scale=-1.0)
    fac = pool.tile([P, w], F32, tag=f"fac{tg}")
    nc.vector.tensor_mul(fac[:], sh[:], rinv[:])
    gfac = pool.tile([P, w], F32, tag=f"gfac{tg}")
    nc.vector.tensor_scalar(gfac[:], fac[:], alpha_b[:], None, op0=ALU.mult)
    # time = sqrt(1/curv + fac^2 * nsq)
    f2n = pool.tile([P, w], F32, tag=f"f2n{tg}")
    nc.vector.tensor_mul(f2n[:], fac[:], fac[:])
    nc.vector.tensor_mul(f2n[:], f2n[:], nsq[:])
    nc.vector.tensor_scalar(f2n[:], f2n[:], icurv_b[:], None, op0=ALU.add)
    t = pool.tile([P, w], F32, tag=f"t{tg}")
    nc.scalar.activation(t[:], f2n[:], AF.Sqrt)
    # tq = t / gfac = Exp(Ln t - Ln gfac)
    lnt = pool.tile([P, w], F32, tag=f"lnt{tg}")
    nc.scalar.activation(lnt[:], t[:], AF.Ln)
    lng = pool.tile([P, w], F32, tag=f"lng{tg}")
    nc.scalar.activation(lng[:], gfac[:], AF.Ln)
    nc.vector.tensor_sub(lnt[:], lnt[:], lng[:])
    tq = pool.tile([P, w], F32, tag=f"tq{tg}")
    nc.scalar.activation(tq[:], lnt[:], AF.Exp)
    return gfac, tq


def _split_hi_lo(nc, pool, x, w, tg):
    """Split f32 (128,w) into exact bf16 hi + lo halves."""
    hi = pool.tile([P, w], BF16, tag=f"hi{tg}")
    nc.vector.tensor_copy(hi[:], x[:])
    hif = pool.tile([P, w], F32, tag=f"hif{tg}")
    nc.vector.tensor_copy(hif[:], hi[:])
    lof = pool.tile([P, w], F32, tag=f"lof{tg}")
    nc.vector.tensor_sub(lof[:], x[:], hif[:])
    lo = pool.tile([P, w], BF16, tag=f"lo{tg}")
    nc.vector.tensor_copy(lo[:], lof[:])
    return hi, lo


def build_kernel(ctx: ExitStack, tc: tile.TileContext, nt_m: int = NT_M):
    nc = tc.nc

    text_d = nc.dram_tensor("text_embeddings", (M_LOC, D), F32, kind="ExternalInput").ap()
    labels_d = nc.dram_tensor("label_embeddings", (C, D), F32, kind="ExternalInput").ap()
    tgt_d = nc.dram_tensor("target_labels", (M_LOC, 1), I32, kind="ExternalInput").ap()
    curv_log_d = nc.dram_tensor("curv_log", (1, 1), F32, kind="ExternalInput").ap()
    ta_log_d = nc.dram_tensor("text_alpha_log", (1, 1), F32, kind="ExternalInput").ap()
    la_log_d = nc.dram_tensor("label_alpha_log", (1, 1), F32, kind="ExternalInput").ap()
    loss_d = nc.dram_tensor("loss", (M_LOC, 1), F32, kind="ExternalOutput").ap()

    const = ctx.enter_context(tc.tile_pool(name="const", bufs=1))
    tiny = ctx.enter_context(tc.tile_pool(name="tiny", bufs=2))
    sc = ctx.enter_context(tc.tile_pool(name="scratch", bufs=2))
    junk = ctx.enter_context(tc.tile_pool(name="junk", bufs=1))
    ypool = ctx.enter_context(tc.tile_pool(name="ypool", bufs=1))
    xpool = ctx.enter_context(tc.tile_pool(name="xpool", bufs=2))
    inner_pool = ctx.enter_context(tc.tile_pool(name="inner", bufs=2))
    psum = ctx.enter_context(tc.tile_pool(name="psum", bufs=8, space="PSUM"))
    dram = ctx.enter_context(tc.tile_pool(name="dram", bufs=1, space="DRAM"))

    # ---- runtime scalars: stride-0 DMA broadcast to (128,1), then derive ----
    def bload(ap_d, tag):
        b = const.tile([P, 1], F32, tag=tag)
        nc.sync.dma_start(b[:], bass.AP(ap_d.tensor, 0, [[0, P], [1, 1]]))
        return b

    cl_b = bload(curv_log_d, "cl_b")
    ta_b = bload(ta_log_d, "ta_b")
    la_b = bload(la_log_d, "la_b")
    curv_b = const.tile([P, 1], F32, tag="curv_b")
    nc.scalar.activation(curv_b[:], cl_b[:], AF.Exp)
    at_b = const.tile([P, 1], F32, tag="at_b")
    nc.scalar.activation(at_b[:], ta_b[:], AF.Exp)
    al_b = const.tile([P, 1], F32, tag="al_b")
    nc.scalar.activation(al_b[:], la_b[:], AF.Exp)
    icurv_b = const.tile([P, 1], F32, tag="icurv_b")
    nc.scalar.activation(icurv_b[:], cl_b[:], AF.Exp, scale=-1.0)
    isqc_b = const.tile([P, 1], F32, tag="isqc_b")
    nc.scalar.activation(isqc_b[:], cl_b[:], AF.Exp, scale=-0.5)
    ncurv_b = const.tile([P, 1], F32, tag="ncurv_b")
    nc.vector.tensor_scalar_mul(ncurv_b[:], curv_b[:], -1.0)

    # ---- constants ----
    iota_f = const.tile([P, C], F32, tag="iota_f")
    nc.gpsimd.iota(
        iota_f[:], [[1, C]], channel_multiplier=0,
        allow_small_or_imprecise_dtypes=True,
    )
    eps24 = const.tile([P, 3 * NT_M], F32, tag="eps24")
    nc.gpsimd.memset(eps24[:], 1.0 + EPS)

    # ---- label pass: raw bf16 scratch + per-label norms (G tiles per load) ----
    G = 2
    yscr = dram.tile([C, D], BF16, tag="yscr")
    nsqy_raw = const.tile([P, NT_C], F32, tag="nsqy_raw")
    for cg in range(NT_C // G):
        r0 = cg * G * P
        src = labels_d[r0:r0 + G * P, :].rearrange("(a p) d -> p a d", p=P)
        lab = sc.tile([P, G, D], F32, tag="lab")
        nc.sync.dma_start(lab[:], src)
        ybf = sc.tile([P, G, D], BF16, tag="ybf")
        nc.vector.tensor_copy(ybf[:], lab[:])
        nc.sync.dma_start(yscr[r0:r0 + G * P, :].rearrange("(a p) d -> p a d", p=P), ybf[:])
        j = junk.tile([P, D], F32, tag="jD")
        for a in range(G):
            nc.scalar.activation(
                j[:], lab[:, a, :], AF.Square,
                accum_out=nsqy_raw[:, cg * G + a:cg * G + a + 1],
            )

    g_col, ytq = _stats_chain(nc, const, nsqy_raw, NT_C, al_b, curv_b, icurv_b, "y")
    ytqn = const.tile([P, NT_C], F32, tag="ytqn")
    nc.vector.tensor_scalar_mul(ytqn[:], ytq[:], -1.0)
    yth, ytl = _split_hi_lo(nc, const, ytqn, NT_C, "y")

    # column scratches (4096,1): strided write from (128,32) column layout
    col_ap = [[1, P], [P, NT_C]]
    gscr = dram.tile([C, 1], F32, tag="gscr")
    nc.sync.dma_start(bass.AP(gscr[:].tensor, 0, col_ap), g_col[:])
    ythscr = dram.tile([C, 1], BF16, tag="ythscr")
    nc.sync.dma_start(bass.AP(ythscr[:].tensor, 0, col_ap), yth[:])
    ytlscr = dram.tile([C, 1], BF16, tag="ytlscr")
    nc.sync.dma_start(bass.AP(ytlscr[:].tensor, 0, col_ap), ytl[:])

    # K-major label tiles via XBAR DMA transpose
    yT = []
    for k in range(KCH):
        t = ypool.tile([P, C], BF16, tag=f"yT{k}")
        nc.sync.dma_start_transpose(t[:], yscr[:, k * P:(k + 1) * P])
        yT.append(t)
    ytrow = ypool.tile([3, C], BF16, tag="ytrow")
    nc.sync.dma_start(ytrow[0:1, :], ythscr[:].rearrange("a b -> b a"))
    nc.sync.dma_start(ytrow[1:2, :], ytlscr[:].rearrange("a b -> b a"))
    nc.sync.dma_start(ytrow[2:3, :], ythscr[:].rearrange("a b -> b a"))
    # g broadcast row (128, C) f32 via stride-0 partition read
    g_bcast = const.tile([P, C], F32, tag="g_bcast")
    nc.sync.dma_start(g_bcast[:], bass.AP(gscr[:].tensor, 0, [[0, P], [1, C]]))

    # ---- text pass: raw bf16 -> DRAM scratch + per-row norms ----
    xscr = dram.tile([M_LOC, D], BF16, tag="xscr")
    nsqx_raw = const.tile([P, NT_M], F32, tag="nsqx_raw")
    for mg in range((nt_m + G - 1) // G):
        r0 = mg * G * P
        gg = min(G, nt_m - mg * G)
        src = text_d[r0:r0 + gg * P, :].rearrange("(a p) d -> p a d", p=P)
        tx = sc.tile([P, G, D], F32, tag="lab")  # share slots with label pass
        nc.sync.dma_start(tx[:, :gg, :], src)
        xbf = sc.tile([P, G, D], BF16, tag="ybf")
        nc.vector.tensor_copy(xbf[:, :gg, :], tx[:, :gg, :])
        nc.sync.dma_start(
            xscr[r0:r0 + gg * P, :].rearrange("(a p) d -> p a d", p=P),
            xbf[:, :gg, :],
        )
        j = junk.tile([P, D], F32, tag="jD")
        for a in range(gg):
            nc.scalar.activation(
                j[:], tx[:, a, :], AF.Square,
                accum_out=nsqx_raw[:, mg * G + a:mg * G + a + 1],
            )
    # K-major text tiles for ALL m-tiles in 6 transpose DMAs
    xT_all = []
    for k in range(KCH):
        t = const.tile([P, M_LOC], BF16, tag=f"xTall{k}")
        nc.sync.dma_start_transpose(t[:], xscr[:, k * P:(k + 1) * P])
        xT_all.append(t)

    hx, xtq = _stats_chain(nc, const, nsqx_raw, NT_M, at_b, curv_b, icurv_b, "x")
    xth, xtl = _split_hi_lo(nc, const, xtq, NT_M, "x")
    xcol_ap = [[1, P], [P, NT_M]]
    xthscr = dram.tile([M_LOC, 1], BF16, tag="xthscr")
    nc.sync.dma_start(bass.AP(xthscr[:].tensor, 0, xcol_ap), xth[:])
    xtlscr = dram.tile([M_LOC, 1], BF16, tag="xtlscr")
    nc.sync.dma_start(bass.AP(xtlscr[:].tensor, 0, xcol_ap), xtl[:])
    xtime3 = const.tile([3, M_LOC], BF16, tag="xtime3")
    nc.sync.dma_start(xtime3[0:1, :], xthscr[:].rearrange("a b -> b a"))
    nc.sync.dma_start(xtime3[1:2, :], xthscr[:].rearrange("a b -> b a"))
    nc.sync.dma_start(xtime3[2:3, :], xtlscr[:].rearrange("a b -> b a"))

    # targets for all m-tiles: (128, NT_M) int32 -> f32
    tgt_all = const.tile([P, NT_M], I32, tag="tgt_all")
    nc.sync.dma_start(tgt_all[:], bass.AP(tgt_d.tensor, 0, [[1, P], [P, NT_M]]))
    tgt_f = const.tile([P, NT_M], F32, tag="tgt_f")
    nc.vector.tensor_copy(tgt_f[:], tgt_all[:])

    V_all = const.tile([P, 3 * NT_M], F32, tag="V_all")

    # ---- main loop over m-tiles ----
    for m in range(nt_m):
        rows = slice(m * P, (m + 1) * P)
        inner = inner_pool.tile([P, C], F32, tag="inner")
        for n in range(NB):
            ns = slice(n * NF, (n + 1) * NF)
            ps = psum.tile([P, NF], F32, tag="ps")
            for k in range(KCH):
                nc.tensor.matmul(
                    ps[:], xT_all[k][:, rows], yT[k][:, ns],
                    start=(k == 0), stop=False,
                )
            nc.tensor.matmul(
                ps[:], xtime3[:, rows], ytrow[:, ns], start=False, stop=True
            )
            # inner = (ps * hx_m) * g  — rescale while copying PSUM -> SBUF
            nc.vector.scalar_tensor_tensor(
                inner[:, ns], ps[:], hx[:, m:m + 1], g_bcast[:, ns],
                op0=ALU.mult, op1=ALU.mult,
            )

        # top-8 candidates
        top8 = tiny.tile([P, 8], F32, tag="top8")
        nc.vector.max(top8[:], inner[:])

        # positive logit: sum((iota == target) * inner) along the row;
        # the elementwise product overwrites inner in place (dead afterwards)
        pos_val = tiny.tile([P, 1], F32, tag="pos_val")
        nc.vector.scalar_tensor_tensor(
            inner[:], iota_f[:], tgt_f[:, m:m + 1], inner[:],
            op0=ALU.is_equal, op1=ALU.mult, accum_out=pos_val[:],
        )

        # knock the positive out of the candidates (one occurrence only)
        repl = tiny.tile([P, 8], F32, tag="repl")
        nc.gpsimd.memset(repl[:], BIG)
        nc.vector.tensor_copy(repl[:, 0:1], pos_val[:])
        masked = tiny.tile([P, 8], F32, tag="masked")
        nc.vector.match_replace(masked[:], repl[:], top8[:], -BIG)
        sorted8 = tiny.tile([P, 8], F32, tag="sorted8")
        nc.vector.max(sorted8[:], masked[:])

        nc.vector.tensor_copy(V_all[:, 3 * m:3 * m + 1], pos_val[:])
        nc.vector.tensor_copy(V_all[:, 3 * m + 1:3 * m + 3], sorted8[:, 0:2])

    # ---- batched loss tail over all m-tiles ----
    W = 3 * nt_m
    cd = const.tile([P, 3 * NT_M], F32, tag="cd")
    nc.vector.scalar_tensor_tensor(
        cd[:, :W], V_all[:, :W], ncurv_b[:], eps24[:, :W],
        op0=ALU.mult, op1=ALU.max,
    )
    sq = const.tile([P, 3 * NT_M], F32, tag="sqv")
    nc.vector.tensor_mul(sq[:, :W], cd[:, :W], cd[:, :W])
    nc.vector.tensor_scalar_add(sq[:, :W], sq[:, :W], -1.0)
    root = const.tile([P, 3 * NT_M], F32, tag="root")
    nc.scalar.activation(root[:, :W], sq[:, :W], AF.Sqrt)
    u = const.tile([P, 3 * NT_M], F32, tag="u")
    nc.vector.tensor_add(u[:, :W], cd[:, :W], root[:, :W])
    dist = const.tile([P, 3 * NT_M], F32, tag="dist")
    nc.scalar.activation(dist[:, :W], u[:, :W], AF.Ln)
    nc.vector.tensor_scalar(dist[:, :W], dist[:, :W], isqc_b[:], None, op0=ALU.mult)
    ev = const.tile([P, 3 * NT_M], F32, tag="ev")
    nc.scalar.activation(ev[:, :W], dist[:, :W], AF.Exp, scale=-1.0)
    s8 = const.tile([P, NT_M], F32, tag="s8")
    ev3 = ev[:, :W].rearrange("p (m k) -> p m k", k=3)
    nc.vector.tensor_reduce(s8[:, :nt_m], ev3, axis=AX.X, op=ALU.add)
    lg = const.tile([P, NT_M], F32, tag="lg")
    nc.scalar.activation(lg[:, :nt_m], s8[:, :nt_m], AF.Ln)
    loss_all = const.tile([P, NT_M], F32, tag="loss_all")
    dist_pos = dist[:, :W].rearrange("p (m k) -> p m k", k=3)[:, :, 0:1]
    nc.vector.tensor_add(
        loss_all[:, :nt_m], lg[:, :nt_m],
        dist_pos.rearrange("p m k -> p (m k)"),
    )
    nc.sync.dma_start(
        bass.AP(loss_d.tensor, 0, [[1, P], [P, nt_m]]), loss_all[:, :nt_m]
    )


_CACHED = {}


def build_program(nt_m: int = NT_M):
    if nt_m not in _CACHED:
        nc = bacc.Bacc(
            "TRN2",
            target_bir_lowering=False,
            debug=False,
            enable_asserts=False,
            num_devices=N_CORES,
        )
        with tile.TileContext(nc) as tc, ExitStack() as ctx:
            build_kernel(ctx, tc, nt_m)
        nc.compile()
        _CACHED[nt_m] = nc
    return _CACHED[nt_m]


def shard_inputs(inputs) -> list[dict[str, np.ndarray]]:
    text = np.ascontiguousarray(np.asarray(inputs["text_embeddings"], np.float32))
    labels = np.ascontiguousarray(np.asarray(inputs["label_embeddings"], np.float32))
    tgt = np.asarray(inputs["target_labels"]).astype(np.int32).reshape(M_FULL, 1)
    s11 = lambda v: np.asarray(v, np.float32).reshape(1, 1)
    curv_log = s11(inputs["curv_log"])
    ta = s11(inputs["text_alpha_log"])
    la = s11(inputs["label_alpha_log"])
    in_maps = []
    for i in range(N_CORES):
        r = slice(i * M_LOC, (i + 1) * M_LOC)
        in_maps.append({
            "text_embeddings": np.ascontiguousarray(text[r]),
            "label_embeddings": labels,
            "target_labels": np.ascontiguousarray(tgt[r]),
            "curv_log": curv_log,
            "text_alpha_log": ta,
            "label_alpha_log": la,
        })
    return in_maps


def run_sharded(inputs, trace=False, nt_m: int = NT_M, **kwargs):
    nc = build_program(nt_m)
    in_maps = shard_inputs(inputs)
    res = bass_utils.run_bass_kernel_spmd(
        nc, in_maps, core_ids=list(range(N_CORES)), trace=trace, **kwargs
    )
    loss = np.concatenate(
        [res.results[i]["loss"].reshape(M_LOC) for i in range(N_CORES)]
    ).astype(np.float32)
    return loss, res


def kernel(**inputs) -> np.ndarray:
    loss, _ = run_sharded(inputs, trace=False)
    return loss
